# revision 2
# baseline (speedup 1.0000x reference)
"""Confusion-matrix (joint histogram) kernel for Trainium2.

Math: out[b, i, j] = #{pixels p in batch b : yp[b,p] == i and y[b,p] == j}
for i, j in [0, 21). Inputs yp, y are [8, 2048, 2048] int32 with values in
[0, 21).

Strategy (per NeuronCore; core c processes batch c):
  - one-hot masks as bf16 {0,1} "class planes" built with DVE
    tensor_scalar(is_equal) (4x perf mode),
  - joint counts via TensorE: confusion = onehot(yp)^T @ onehot(y),
    blocked 6 pixel-columns at a time (lhsT/rhs [128, 6*21]) accumulating
    into a single PSUM [126, 126] f32 tile (exact: integer counts < 2^24),
  - host extracts + sums the 6 diagonal 21x21 blocks.
"""

import os

import numpy as np

C = 21                  # classes
G = 6                   # pixel-column groups per matmul (G*C = 126 <= 128)
M = G * C               # 126
P = 128                 # partitions
FP = 504                # plane-chunk columns (divisible by 6)
DMA_GROUP = 5           # plane-chunks per DMA chunk
SENTINEL = 64.0         # bf16-exact value outside [0, 21)

_CACHE = {}


def _build(n_free):
    import concourse.bacc as bacc
    import concourse.mybir as mybir
    import concourse.tile as tile

    nc = bacc.Bacc(
        "TRN2",
        target_bir_lowering=False,
        debug=False,
        enable_asserts=False,
        num_devices=8,
    )
    yp = nc.dram_tensor("yp", [P, n_free], mybir.dt.int32, kind="ExternalInput").ap()
    y = nc.dram_tensor("y", [P, n_free], mybir.dt.int32, kind="ExternalInput").ap()
    out = nc.dram_tensor("out", [M, M], mybir.dt.float32, kind="ExternalOutput").ap()

    n_main = (n_free // FP) * FP
    tail_cols = n_free - n_main                      # < FP
    tail_pad = -tail_cols % G                        # pad to multiple of 6
    tail_w = tail_cols + tail_pad
    total_mms = (n_main // G) + (tail_w // G)

    bf16 = mybir.dt.bfloat16
    f32 = mybir.dt.float32
    i32 = mybir.dt.int32

    with tile.TileContext(nc) as tc:
        with (
            tc.tile_pool(name="psum", bufs=1, space="PSUM") as psum_pool,
            tc.tile_pool(name="dma", bufs=2) as dma_pool,
            tc.tile_pool(name="cat", bufs=2) as cat_pool,
            tc.tile_pool(name="planes", bufs=2) as plane_pool,
            tc.tile_pool(name="singles", bufs=1) as singles,
        ):
            acc = psum_pool.tile([M, M], f32)
            mm = 0

            def do_plane_chunk(cat, w):
                """cat: [128, 2*w] bf16 = [yp vals | y vals]; w divisible by 6.

                Plane layout is matmul-ready: planes[p, blk*126 + i*6 + g]
                = (cat[p, blk*6+g] == i), blk in [0, 2*w/6). A-side tiles are
                blks [0, w/6), B-side blks [w/6, 2*w/6). Each matmul reads a
                contiguous [128, 126] slice (single free dim, required by
                the BIR verifier).
                """
                nonlocal mm
                nblk = 2 * w // G
                planes = plane_pool.tile([P, C * 2 * FP], bf16, tag="planes")
                pl3 = planes[:, : nblk * M].rearrange("p (b f) -> p b f", f=M)
                cat3 = cat[:].rearrange("p (b f) -> p b f", f=G)
                for i in range(C):
                    nc.vector.tensor_scalar(
                        pl3[:, :, i * G : (i + 1) * G],
                        cat3[:],
                        float(i),
                        None,
                        mybir.AluOpType.is_equal,
                    )
                half = (w // G) * M
                for t in range(w // G):
                    nc.tensor.matmul(
                        acc[:, :],
                        planes[:, t * M : (t + 1) * M],
                        planes[:, half + t * M : half + (t + 1) * M],
                        start=(mm == 0),
                        stop=(mm == total_mms - 1),
                    )
                    mm += 1

            # main span: DMA chunks of DMA_GROUP plane-chunks
            n_chunks = n_main // FP
            off = 0
            while off < n_main:
                nsub = min(DMA_GROUP, (n_main - off) // FP)
                w = nsub * FP
                yp32 = dma_pool.tile([P, DMA_GROUP * FP], i32, tag="yp32")
                y32 = dma_pool.tile([P, DMA_GROUP * FP], i32, tag="y32")
                nc.sync.dma_start(yp32[:, :w], yp[:, off : off + w])
                nc.sync.dma_start(y32[:, :w], y[:, off : off + w])
                for k in range(nsub):
                    cat = cat_pool.tile([P, 2 * FP], bf16, tag="cat")
                    nc.scalar.copy(cat[:, :FP], yp32[:, k * FP : (k + 1) * FP])
                    nc.scalar.copy(cat[:, FP:], y32[:, k * FP : (k + 1) * FP])
                    do_plane_chunk(cat, FP)
                off += w

            # tail: tail_cols real columns + tail_pad sentinel columns
            if tail_cols:
                ypt = dma_pool.tile([P, DMA_GROUP * FP], i32, tag="yp32")
                yt = dma_pool.tile([P, DMA_GROUP * FP], i32, tag="y32")
                nc.sync.dma_start(ypt[:, :tail_cols], yp[:, n_main:])
                nc.sync.dma_start(yt[:, :tail_cols], y[:, n_main:])
                catt = cat_pool.tile([P, 2 * FP], bf16, tag="cat")
                ct = catt[:, : 2 * tail_w]
                if tail_pad:
                    nc.vector.memset(ct[:], SENTINEL)
                nc.scalar.copy(ct[:, :tail_cols], ypt[:, :tail_cols])
                nc.scalar.copy(
                    ct[:, tail_w : tail_w + tail_cols], yt[:, :tail_cols]
                )
                do_plane_chunk(ct, tail_w)

            assert mm == total_mms
            res = singles.tile([M, M], f32)
            nc.vector.tensor_copy(res[:], acc[:, :])
            nc.sync.dma_start(out, res[:])

    nc.compile()
    return nc


def _get(n_free):
    if n_free not in _CACHE:
        _CACHE[n_free] = _build(n_free)
    return _CACHE[n_free]


def kernel(yp, y, res, n_classes, _trace=False):
    from concourse import bass_utils

    yp = np.ascontiguousarray(np.asarray(yp))
    y = np.ascontiguousarray(np.asarray(y))
    B = yp.shape[0]
    n_free = yp[0].size // P
    nc = _get(n_free)
    in_maps = [
        {"yp": yp[b].reshape(P, n_free), "y": y[b].reshape(P, n_free)}
        for b in range(B)
    ]
    r = bass_utils.run_bass_kernel_spmd(
        nc, in_maps, core_ids=list(range(B)), trace=_trace
    )
    outs = []
    for b in range(B):
        Pm = r.results[b]["out"].astype(np.float64)
        Rb = np.zeros((C, C), np.float64)
        for g in range(G):
            Rb += Pm[g::G, g::G]
        outs.append(Rb)
    res_np = np.stack(outs).astype(np.float32)
    if _trace:
        kernel._last_results = r
    return res_np
